# revision 2
# baseline (speedup 1.0000x reference)
"""MoE feed-forward block (shared expert + top-2-of-8 routed experts) on 8
Trainium2 NeuronCores.

Sharding: expert-parallel with host-side token dispatch (the shard step).
The host computes the top-2 routing (gate logits + softmax over the selected
pair) and gathers, for core c, the tokens that routed to expert c (padded to a
uniform capacity `cap`) plus a 512-token slice of the shared expert's work.
Each core then runs the SAME program (SPMD): a plain FFN
    y = (silu(x @ w1.T) @ w2.T) * g[token]
over its token buffer, where the first `cap` tokens use expert c's weights and
per-token gate values g, and the last 512 tokens use the shared-expert weights
with g=1. The host scatter-adds the per-core outputs into the full result (the
unshard step). This does 1/3 of the dense-MoE FLOPs (3 effective experts per
token instead of 9).

Matmuls run in bf16 with fp32 accumulation in PSUM.

Device layout (all [*, token]-major so mm1's silu output feeds mm2 directly):
  mm1: h.T[H,Tc]  = w1T[D,H].T @ x.T[D,Tc]      (lhsT=w1T stationary)
  mm2: y[Tc,D]    = sh.T[H,Tc].T @ w2T[H,D]     (lhsT=sh.T stationary)
gate coefficient applied per-partition (token) on the mm2 PSUM via ACT scale.
"""

import ml_dtypes
import numpy as np

import concourse.bass as bass
import concourse.mybir as mybir
import concourse.tile as tile
from concourse import bacc
from concourse.bass import ds, ts
from concourse.bass_utils import run_bass_kernel_spmd

BF16 = ml_dtypes.bfloat16

D_MODEL = 1024
HIDDEN = 4096
N_EXP = 8
N_CORES = 8
T = 4096                      # 2 * 2048 tokens
SH = T // N_CORES             # shared-expert tokens per core
TC = 512                      # max token chunk (one PSUM bank at fp32)
P = 128

LAST_EXEC_NS = None
LAST_RESULT = None


def _chunks(cap):
    """Split the token buffer [cap routed | SH shared] into (off, size,
    is_shared) chunks of at most TC tokens, multiples of P."""
    out = []
    off = 0
    while off < cap:
        sz = min(TC, cap - off)
        out.append((off, sz, False))
        off += sz
    while off < cap + SH:
        sz = min(TC, cap + SH - off)
        out.append((off, sz, True))
        off += sz
    return out


def _build_nc(M, chunks):
    fp32 = mybir.dt.float32
    bf16 = mybir.dt.bfloat16
    AF = mybir.ActivationFunctionType

    nc = bacc.Bacc()
    xbf = nc.declare_dram_parameter("xbf", [P, 8, M], bf16, isOutput=False)
    w1e = nc.declare_dram_parameter("w1e", [P, 8, HIDDEN], bf16, isOutput=False)
    w2e = nc.declare_dram_parameter("w2e", [P, 32, D_MODEL], bf16, isOutput=False)
    w1s = nc.declare_dram_parameter("w1s", [P, 8, HIDDEN], bf16, isOutput=False)
    w2s = nc.declare_dram_parameter("w2s", [P, 32, D_MODEL], bf16, isOutput=False)
    gw = nc.declare_dram_parameter("g", [P, M // P], fp32, isOutput=False)
    out = nc.declare_dram_parameter("out", [M, D_MODEL], fp32, isOutput=True)

    with tile.TileContext(nc) as tc:
        with (
            tc.tile_pool(name="const", bufs=1) as cpool,
            tc.tile_pool(name="w2p", bufs=1) as w2pool,
            tc.tile_pool(name="w1s_", bufs=2) as w1pool,
            tc.tile_pool(name="xs", bufs=1) as xpool,
            tc.tile_pool(name="shp", bufs=1) as shpool,
            tc.tile_pool(name="outp", bufs=3) as opool,
            tc.tile_pool(name="ps1", bufs=2, space="PSUM") as pspool,
            tc.tile_pool(name="ps2", bufs=2, space="PSUM") as ps2pool,
        ):
            # Per-k-tile DMAs throughout: one big strided DMA fans out across
            # many HW-DGE queues, and the first consuming matmul then needs
            # more sync-wait slots than walrus allows. Per-k transfers keep
            # each consumer waiting on a single queue semaphore.
            g_sb = cpool.tile([P, M // P], fp32, tag="g")
            nc.sync.dma_start(g_sb[:], gw[:])

            # all tokens' activations stay resident ([P, 8, M] bf16)
            xall = xpool.tile([P, 8, M], bf16, tag="x")
            for k in range(8):
                nc.sync.dma_start(xall[:, k, :], xbf[:, k, :])

            # w2 for the expert segment; re-loaded (same buffer) with the
            # shared-expert w2 once the expert chunks' mm2 is done.
            w2_sb = w2pool.tile([P, 32, D_MODEL], bf16, tag="w2")
            for k in range(32):
                nc.sync.dma_start(w2_sb[:, k, :], w2e[:, k, :])

            seen_shared = False
            for (off, sz, shared) in chunks:
                w1src = w1s if shared else w1e
                if shared and not seen_shared:
                    seen_shared = True
                    w2_sb = w2pool.tile([P, 32, D_MODEL], bf16, tag="w2")
                    for k in range(32):
                        nc.sync.dma_start(w2_sb[:, k, :], w2s[:, k, :])

                # ---- mm1 + silu: sh.T[H, sz] ----
                shT = shpool.tile([P, HIDDEN // P, TC], bf16, tag="shT")
                for ht in range(HIDDEN // P):
                    if ht % 4 == 0:
                        w1tile = w1pool.tile([P, 8, 512], bf16, tag="w1")
                        for k in range(8):
                            nc.sync.dma_start(w1tile[:, k, :],
                                              w1src[:, k, ds(ht * P, 512)])
                    ph = pspool.tile([P, sz], fp32, tag="ph")
                    for k in range(8):
                        nc.tensor.matmul(ph[:], w1tile[:, k, ts(ht % 4, P)],
                                         xall[:, k, ds(off, sz)],
                                         start=(k == 0), stop=(k == 7))
                    nc.scalar.activation(shT[:, ht, :sz], ph[:], AF.Silu)

                # ---- mm2: y[sz, D] = sh.T @ w2T, scaled per-token ----
                for mt in range(sz // P):
                    tt = off // P + mt
                    for nh in range(D_MODEL // 512):
                        py = ps2pool.tile([P, 512], fp32, tag="py")
                        for k in range(HIDDEN // P):
                            nc.tensor.matmul(py[:], shT[:, k, ts(mt, P)],
                                             w2_sb[:, k, ts(nh, 512)],
                                             start=(k == 0),
                                             stop=(k == HIDDEN // P - 1))
                        ysb = opool.tile([P, 512], fp32, tag="ysb")
                        nc.scalar.activation(ysb[:], py[:], AF.Copy,
                                             scale=g_sb[:, tt:tt + 1])
                        nc.sync.dma_start(out[ds(tt * P, P), ds(nh * 512, 512)],
                                          ysb[:])
    nc.compile()
    return nc


def _strip(a, dtype):
    # [K, F] -> [128, K//128, F] partition-major layout
    k, f = a.shape
    return np.ascontiguousarray(
        a.reshape(k // P, P, f).transpose(1, 0, 2)).astype(dtype)


def kernel(x, shared_w1, shared_w2, experts_w1, experts_w2, gate_w):
    global LAST_EXEC_NS, LAST_RESULT
    x = np.asarray(x, dtype=np.float32).reshape(T, D_MODEL)
    shared_w1 = np.asarray(shared_w1, dtype=np.float32)
    shared_w2 = np.asarray(shared_w2, dtype=np.float32)
    experts_w1 = np.asarray(experts_w1, dtype=np.float32)
    experts_w2 = np.asarray(experts_w2, dtype=np.float32)
    gate_w = np.asarray(gate_w, dtype=np.float32)

    # ---- host-side top-2 routing (the dispatch/shard step) ----
    z = x @ gate_w.T                                    # [T, E] fp32
    ar = np.arange(T)
    i1 = np.argmax(z, axis=1)
    zm = z.copy()
    zm[ar, i1] = -np.inf
    i2 = np.argmax(zm, axis=1)
    z1 = z[ar, i1].astype(np.float64)
    z2 = z[ar, i2].astype(np.float64)
    e2 = np.exp(z2 - z1)
    g1 = (1.0 / (1.0 + e2)).astype(np.float32)
    g2 = (e2 / (1.0 + e2)).astype(np.float32)

    idx_lists, gv_lists = [], []
    for e in range(N_EXP):
        m1 = i1 == e
        m2 = i2 == e
        idx = np.nonzero(m1 | m2)[0]
        gv = np.where(m1, g1, g2)[idx]
        idx_lists.append(idx)
        gv_lists.append(gv)

    n_max = max(len(ix) for ix in idx_lists)
    cap = max(P, ((n_max + P - 1) // P) * P)
    M = cap + SH

    sw1t = _strip(np.ascontiguousarray(shared_w1.T), BF16)   # [128, 8, H]
    sw2t = _strip(np.ascontiguousarray(shared_w2.T), BF16)   # [128, 32, D]

    in_maps = []
    for c in range(N_CORES):
        idx, gv = idx_lists[c], gv_lists[c]
        n = len(idx)
        xc = np.zeros((M, D_MODEL), dtype=np.float32)
        xc[:n] = x[idx]
        xc[cap:] = x[c * SH:(c + 1) * SH]
        gfull = np.zeros(M, dtype=np.float32)
        gfull[:n] = gv
        gfull[cap:] = 1.0

        in_maps.append({
            "xbf": _strip(np.ascontiguousarray(xc.T), BF16),  # [128, 8, M]
            "w1e": _strip(np.ascontiguousarray(experts_w1[c].T), BF16),
            "w2e": _strip(np.ascontiguousarray(experts_w2[c].T), BF16),
            "w1s": sw1t, "w2s": sw2t,
            "g": np.ascontiguousarray(gfull.reshape(M // P, P).T),
        })

    nc = _build_nc(M, _chunks(cap))
    res = run_bass_kernel_spmd(nc, in_maps, list(range(N_CORES)))
    LAST_EXEC_NS = res.exec_time_ns
    LAST_RESULT = res

    out = np.zeros((T, D_MODEL), dtype=np.float32)
    for c in range(N_CORES):
        y = res.results[c]["out"]                         # [M, D] fp32
        out[c * SH:(c + 1) * SH] = y[cap:]
    for c in range(N_CORES):
        y = res.results[c]["out"]
        idx = idx_lists[c]
        out[idx] += y[:len(idx)]
    return out.reshape(2, 2048, D_MODEL).astype(np.float32)


# revision 4
# speedup vs baseline: 1.1991x; 1.1991x over previous
"""MoE feed-forward block (shared expert + top-2-of-8 routed experts) on 8
Trainium2 NeuronCores.

Sharding: expert-parallel with host-side token dispatch (the shard step).
The host computes the top-2 routing and gathers, for core c, the tokens that
routed to expert c (padded to a uniform capacity `cap`) plus a 512-token slice
of the shared expert's work. Each core runs the SAME program (SPMD): a plain
FFN  y = silu(x @ w1.T) @ w2.T  over its token buffer, where the first `cap`
tokens use expert c's weights and the last 512 use the shared-expert weights.
The host applies the per-token gate coefficients while scatter-adding the
per-core outputs into the full result (the unshard step). This does 1/3 of
the dense-MoE FLOPs (3 effective experts per token instead of 9).

Matmuls run in bf16 with fp32 accumulation in PSUM.

Device layout (everything [feature, token]-major, 128-partition stripped):
  mm1: h.T[H,t]  = w1T[D,H].T @ x.T[D,t]     (lhsT=w1T stationary)
  mm2: y.T[D,t]  = w2T[H,D].T @ sh.T[H,t]    (lhsT=w2T stationary)
Each stationary load is shared by the matmuls of all token sub-chunks of the
segment (LDWEIGHTS amortized), and mm2's moving operand is the silu output so
no transposes are needed anywhere.
"""

import ml_dtypes
import numpy as np

import concourse.bass as bass
import concourse.mybir as mybir
import concourse.tile as tile
from concourse import bacc
from concourse.bass import ds, ts
from concourse.bass_utils import run_bass_kernel_spmd

BF16 = ml_dtypes.bfloat16

D_MODEL = 1024
HIDDEN = 4096
N_EXP = 8
N_CORES = 8
T = 4096                      # 2 * 2048 tokens
SH = T // N_CORES             # shared-expert tokens per core
P = 128

LAST_EXEC_NS = None
LAST_RESULT = None


def _split(n):
    """Split n tokens into sub-chunks (multiples of P, <=512, prefer >=2
    pieces so consecutive matmuls share each LDWEIGHTS)."""
    if n <= 128:
        return [n]
    if n <= 512:
        a = (n // 2 + P - 1) // P * P
        return [a, n - a]
    out = []
    while n > 512:
        out.append(512)
        n -= 512
    if n:
        out.append(n)
    return out


def _build_nc(M, cap):
    fp32 = mybir.dt.float32
    bf16 = mybir.dt.bfloat16
    AF = mybir.ActivationFunctionType

    nc = bacc.Bacc()
    xbf = nc.declare_dram_parameter("xbf", [P, 8, M], bf16, isOutput=False)
    w1e = nc.declare_dram_parameter("w1e", [P, 8, HIDDEN], bf16, isOutput=False)
    w2e = nc.declare_dram_parameter("w2e", [P, 32, D_MODEL], bf16, isOutput=False)
    w1s = nc.declare_dram_parameter("w1s", [P, 8, HIDDEN], bf16, isOutput=False)
    w2s = nc.declare_dram_parameter("w2s", [P, 32, D_MODEL], bf16, isOutput=False)
    outT = nc.declare_dram_parameter("outT", [P, 8, M], bf16, isOutput=True)

    # (segment base, sub-chunk sizes, weight params) — expert first, shared
    # second; sub-chunk offsets are segment-relative.
    segs = [(0, _split(cap), w1e, w2e), (cap, _split(SH), w1s, w2s)]

    with tile.TileContext(nc) as tc:
        with (
            tc.tile_pool(name="xs", bufs=1) as xpool,
            tc.tile_pool(name="w1p", bufs=2) as w1pool,
            tc.tile_pool(name="w2p", bufs=1) as w2pool,
            tc.tile_pool(name="shp", bufs=1) as shpool,
            tc.tile_pool(name="outp", bufs=3) as opool,
            tc.tile_pool(name="ps", bufs=2, space="PSUM") as pspool,
        ):
            # Per-k-tile DMAs throughout: keeps each consumer waiting on a
            # single queue semaphore (walrus sync-wait slot limit).
            xall = xpool.tile([P, 8, M], bf16, tag="x")
            for k in range(8):
                nc.sync.dma_start(xall[:, k, 0:cap], xbf[:, k, 0:cap])

            for (base, sizes, w1src, w2src) in segs:
                offs = np.cumsum([0] + sizes[:-1]).tolist()
                if base != 0:  # shared segment: x slice needed only now
                    for k in range(8):
                        nc.sync.dma_start(xall[:, k, base:base + SH],
                                          xbf[:, k, base:base + SH])

                # ---- mm1 + silu: sh.T[H, seg] ----
                shT = shpool.tile([P, HIDDEN // P, cap], bf16, tag="shT")
                for ht in range(HIDDEN // P):
                    if ht % 4 == 0:
                        w1tile = w1pool.tile([P, 8, 512], bf16, tag="w1")
                        for k in range(8):
                            nc.sync.dma_start(w1tile[:, k, :],
                                              w1src[:, k, ds(ht * P, 512)])
                    phs = [pspool.tile([P, 512], fp32, tag=f"ph{j}",
                                       name=f"ph{j}")
                           for j in range(len(sizes))]
                    for k in range(8):
                        for j, (off, sz) in enumerate(zip(offs, sizes)):
                            nc.tensor.matmul(phs[j][:, :sz],
                                             w1tile[:, k, ts(ht % 4, P)],
                                             xall[:, k, ds(base + off, sz)],
                                             start=(k == 0), stop=(k == 7))
                    for j, (off, sz) in enumerate(zip(offs, sizes)):
                        nc.scalar.activation(shT[:, ht, ds(off, sz)],
                                             phs[j][:, :sz], AF.Silu)

                # w2 in two D-halves: halves the reload WAR window between
                # the expert and shared segments.
                w2h = []
                for h in range(2):
                    w2t = w2pool.tile([P, 32, 512], bf16, tag=f"w2_{h}")
                    for k in range(32):
                        nc.sync.dma_start(w2t[:, k, :],
                                          w2src[:, k, ds(h * 512, 512)])
                    w2h.append(w2t)

                # ---- mm2 (transposed): y.T[D, seg] = w2T.T @ sh.T ----
                for dt in range(D_MODEL // P):
                    w2t = w2h[dt // 4]
                    phs = [pspool.tile([P, 512], fp32, tag=f"ph{j}",
                                       name=f"ph{j}")
                           for j in range(len(sizes))]
                    for k in range(HIDDEN // P):
                        for j, (off, sz) in enumerate(zip(offs, sizes)):
                            nc.tensor.matmul(phs[j][:, :sz],
                                             w2t[:, k, ts(dt % 4, P)],
                                             shT[:, k, ds(off, sz)],
                                             start=(k == 0),
                                             stop=(k == HIDDEN // P - 1))
                    for j, (off, sz) in enumerate(zip(offs, sizes)):
                        ysb = opool.tile([P, 512], bf16, tag="ysb")
                        nc.scalar.activation(ysb[:, :sz], phs[j][:, :sz],
                                             AF.Copy)
                        nc.sync.dma_start(outT[:, dt, ds(base + off, sz)],
                                          ysb[:, :sz])
    nc.compile()
    return nc


def _strip(a, dtype):
    # [K, F] -> [128, K//128, F] partition-major layout
    k, f = a.shape
    return np.ascontiguousarray(
        a.reshape(k // P, P, f).transpose(1, 0, 2)).astype(dtype)


def kernel(x, shared_w1, shared_w2, experts_w1, experts_w2, gate_w):
    global LAST_EXEC_NS, LAST_RESULT
    x = np.asarray(x, dtype=np.float32).reshape(T, D_MODEL)
    shared_w1 = np.asarray(shared_w1, dtype=np.float32)
    shared_w2 = np.asarray(shared_w2, dtype=np.float32)
    experts_w1 = np.asarray(experts_w1, dtype=np.float32)
    experts_w2 = np.asarray(experts_w2, dtype=np.float32)
    gate_w = np.asarray(gate_w, dtype=np.float32)

    # ---- host-side top-2 routing (the dispatch/shard step) ----
    z = x @ gate_w.T                                    # [T, E] fp32
    ar = np.arange(T)
    i1 = np.argmax(z, axis=1)
    zm = z.copy()
    zm[ar, i1] = -np.inf
    i2 = np.argmax(zm, axis=1)
    z1 = z[ar, i1].astype(np.float64)
    z2 = z[ar, i2].astype(np.float64)
    e2 = np.exp(z2 - z1)
    g1 = (1.0 / (1.0 + e2)).astype(np.float32)
    g2 = (e2 / (1.0 + e2)).astype(np.float32)

    idx_lists, gv_lists = [], []
    for e in range(N_EXP):
        m1 = i1 == e
        m2 = i2 == e
        idx = np.nonzero(m1 | m2)[0]
        gv = np.where(m1, g1, g2)[idx]
        idx_lists.append(idx)
        gv_lists.append(gv)

    n_max = max(len(ix) for ix in idx_lists)
    cap = max(P, ((n_max + P - 1) // P) * P)
    M = cap + SH

    sw1t = _strip(np.ascontiguousarray(shared_w1.T), BF16)   # [128, 8, H]
    sw2t = _strip(np.ascontiguousarray(shared_w2.T), BF16)   # [128, 32, D]

    in_maps = []
    for c in range(N_CORES):
        idx = idx_lists[c]
        n = len(idx)
        xc = np.zeros((M, D_MODEL), dtype=np.float32)
        xc[:n] = x[idx]
        xc[cap:] = x[c * SH:(c + 1) * SH]
        in_maps.append({
            "xbf": _strip(np.ascontiguousarray(xc.T), BF16),  # [128, 8, M]
            "w1e": _strip(np.ascontiguousarray(experts_w1[c].T), BF16),
            "w2e": _strip(np.ascontiguousarray(experts_w2[c].T), BF16),
            "w1s": sw1t, "w2s": sw2t,
        })

    nc = _build_nc(M, cap)
    res = run_bass_kernel_spmd(nc, in_maps, list(range(N_CORES)))
    LAST_EXEC_NS = res.exec_time_ns
    LAST_RESULT = res

    out = np.zeros((T, D_MODEL), dtype=np.float32)
    ys = []
    for c in range(N_CORES):
        yT = np.asarray(res.results[c]["outT"], dtype=np.float32)
        y = yT.transpose(1, 0, 2).reshape(D_MODEL, M).T    # [M, D]
        ys.append(y)
        out[c * SH:(c + 1) * SH] = y[cap:]
    for c in range(N_CORES):
        idx, gv = idx_lists[c], gv_lists[c]
        out[idx] += gv[:, None] * ys[c][:len(idx)]
    return out.reshape(2, 2048, D_MODEL).astype(np.float32)
